# revision 77
# baseline (speedup 1.0000x reference)
"""Trainium2 Bass kernel for 2-layer GAT (nn_GAT_4861902979553).

Strategy (8 NeuronCores, SPMD):
  - Nodes sharded contiguously: core c owns rows [c*6250, (c+1)*6250).
  - Edges (incl. self-loops) partitioned by destination core, sorted by dst,
    grouped into 128-dst blocks; each block's edges are packed into 128-edge
    tiles that accumulate into a per-block PSUM via one-hot(alpha) matmuls.
  - Dense phase per layer computes an augmented row per node:
    L1 [h0 | 1 | h1 | 1 | a_src0 | a_src1 | pad] (768B),
    L2 [h2 | 1 | a_src2 | pad] (512B), plus an a_dst side table (fp32 pairs
    bit-packed in an fp16 table).
  - The slab AllGather is split into two half-chunks (sub-half of every
    core's slab, Shared outputs) so the collective pipelines with edge
    compute. Edges are split into three streams: local (src core == dst
    core, gathered from the local slab with no collective dependency —
    overlaps the first collective chunk) and two remote streams keyed by
    the source's sub-half. The edge phase runs stream-major in three
    phases: local flushes per-block fp16 partials to SBUF, phase A
    accumulates into them while the second chunk is in flight, phase B
    re-accumulates in PSUM and the epilogue merges partials + PSUM.
  - The L1 epilogue PE-transposes each block straight into the resident
    layer-2 lhsT (reusing xT's SBUF) and runs layer-2's dense matmuls per
    block, so the second AllGather's first chunk launches mid-edge1.
  - Per-edge rows fetched with dma_gather, 8 tiles (1024 idxs, the SWDGE
    ucode limit) per call, rotating across 4 SWDGE queues so ring drains
    overlap descriptor generation; int16 indices are relative to the
    half-chunk tables (max 8*3125 = 25000).
  - Attention: alpha = exp(lrelu(asrc+adst) - 8); the -8 shift keeps exp in
    fp16 range and cancels in softmax normalization.

Cost-model timeline estimate (sim.py): ~1.40 ms/core vs 1.65 ms for the
serial-collective baseline; both AllGathers except the first chunk are
hidden behind edge compute.
"""

import numpy as np

import os as _os

# Problem constants (hardcoded per harness contract)
N_NODES = 50000
N_EDGES = 800000
IN_FEATS = 256
HIDDEN = 128
NEG_SLOPE = 0.2
N_CORES = 8
P = 128
NSH = N_NODES // N_CORES  # 6250 local rows per core
NCHUNK = 2  # collective half-chunks / edge streams
CSZ = NSH // NCHUNK  # 3125 rows per chunk per core
SHIFT = 8.0  # exp shift; cancels in softmax, keeps fp16 in range
G_TILES = int(_os.environ.get("K_GTILES", "8"))  # edge tiles per gather group (1024-idx SWDGE ucode limit)
PACK2 = int(_os.environ.get("K_PACK2", "0"))  # packed cc2 + repack
PACK1 = int(_os.environ.get("K_PACK1", "0"))  # packed cc1 + repack
ROW1 = 384  # layer-1 gather row (260 used, padded to 768B)
ROW2 = 256  # layer-2 gather row (130 used, padded to 512B)

F16 = np.float16


# --------------------------------------------------------------------------
# Host-side planning
# --------------------------------------------------------------------------

def _wrap_idx(flat):
    """dma_gather index layout: idxs[p, s] = flat[s*16 + p], replicated x8."""
    wrap = flat.reshape(-1, 16).T
    return np.tile(wrap, (8, 1)).astype(np.int16)


def _plan_edges(edge_index, n_nodes, n_cores=N_CORES, g_tiles=G_TILES):
    nsh = n_nodes // n_cores
    nblk = (nsh + P - 1) // P
    src = np.asarray(edge_index[0], np.int64)
    dst = np.asarray(edge_index[1], np.int64)
    loop = np.arange(n_nodes, dtype=np.int64)
    src = np.concatenate([src, loop])
    dst = np.concatenate([dst, loop])
    core = dst // nsh

    # stream 0 = local (src core == dst core; gathers from the local slab)
    # stream 1+j = remote, sub-half j of the source node's core-local row
    # (gathers from the AllGathered chunk-j table)
    def edge_stream(s, d):
        lr = s % nsh
        return np.where(s // nsh == d // nsh, 0, 1 + lr // CSZ)

    def gather_idx(s, st):
        lr = s % nsh
        return np.where(st == 0, lr, (s // nsh) * CSZ + lr - (st - 1) * CSZ)

    NS = NCHUNK + 1  # streams

    # per (core, block, stream) sorted edge lists
    counts = np.zeros((n_cores, nblk, NS), np.int64)
    ecore = []
    for c in range(n_cores):
        m = core == c
        s_c = src[m]
        d_c = dst[m] - c * nsh
        st = edge_stream(s_c, dst[m])
        key = (d_c // P) * NS + st  # sort by (block, stream), then dst
        o = np.lexsort((d_c, key))
        s_c, d_c, st = s_c[o], d_c[o], st[o]
        bh = np.bincount((d_c // P) * NS + st, minlength=nblk * NS)
        counts[c] = bh.reshape(nblk, NS)
        ecore.append((s_c, d_c, st))

    # tiles per (block, stream): max over cores, at least 1 (so that every
    # block has a PSUM chain in every phase)
    tiles_bs = np.maximum(1, (-(-counts // P)).max(axis=0))  # [nblk, NS]
    tiles_pb = tiles_bs.sum(1)
    T = int(tiles_pb.sum())
    blk_start = np.concatenate([[0], np.cumsum(tiles_pb)])[:-1].astype(int)

    # static per-tile structure (identical on every core)
    strm = np.zeros(T, np.int64)
    for b in range(nblk):
        t0 = blk_start[b]
        for s in range(NS):
            strm[t0:t0 + tiles_bs[b, s]] = s
            t0 += tiles_bs[b, s]
    stream_tiles = [np.nonzero(strm == s)[0] for s in range(NS)]
    t_sizes = [len(st) for st in stream_tiles]
    stream_pos = np.zeros(T, np.int64)
    for s in range(NS):
        stream_pos[stream_tiles[s]] = np.arange(t_sizes[s])

    gsrc = np.zeros((n_cores, P, T), np.int64)
    dstcol = np.full((n_cores, P, T), -1.0, np.float32)
    adst_slot = np.zeros((n_cores, P, T), np.int64)
    for c in range(n_cores):
        s_c, d_c, st_c = ecore[c]
        sob = np.concatenate([[0], np.cumsum(counts[c].ravel())]).astype(int)
        for b in range(nblk):
            for s in range(NS):
                e0, e1 = sob[b * NS + s], sob[b * NS + s + 1]
                cnt = e1 - e0
                if cnt == 0:
                    continue
                t0 = blk_start[b] + int(tiles_bs[b, :s].sum())
                o = np.arange(cnt)
                tt, pp = t0 + o // P, o % P
                gsrc[c, pp, tt] = gather_idx(s_c[e0:e1], st_c[e0:e1])
                dstcol[c, pp, tt] = (d_c[e0:e1] - b * P).astype(np.float32)
                adst_slot[c, pp, tt] = d_c[e0:e1]

    # per-stream wrapped int16 index arrays, grouped per g_tiles
    gsrc_w, adsti_w = [], []
    for s in range(NS):
        st = stream_tiles[s]
        gcols, acols = [], []
        for g0 in range(0, len(st), g_tiles):
            tsel = st[g0:g0 + g_tiles]
            gs = gsrc[:, :, tsel]  # [c, P, gw]
            ad = adst_slot[:, :, tsel]
            gw = len(tsel)
            # flat index i = t_rel*128 + p
            gflat = gs.transpose(0, 2, 1).reshape(n_cores, gw * P)
            aflat = ad.transpose(0, 2, 1).reshape(n_cores, gw * P)
            gcols.append(np.stack([_wrap_idx(gflat[c]) for c in range(n_cores)]))
            acols.append(np.stack([_wrap_idx(aflat[c]) for c in range(n_cores)]))
        if gcols:
            gsrc_w.append(np.concatenate(gcols, axis=2))
            adsti_w.append(np.concatenate(acols, axis=2))
        else:
            gsrc_w.append(np.zeros((n_cores, P, 0), np.int16))
            adsti_w.append(np.zeros((n_cores, P, 0), np.int16))

    return dict(
        nsh=nsh, nblk=nblk, T=T, t_sizes=t_sizes,
        tiles_bs=tiles_bs.astype(int), tiles_pb=tiles_pb.astype(int),
        blk_start=blk_start, strm=strm, stream_pos=stream_pos,
        gsrc_w=gsrc_w, adsti_w=adsti_w, dstcol=dstcol,
    )


def _prep_weights(W1, att_src1, att_dst1, W2, att_src2, att_dst2):
    W1t = np.asarray(W1, np.float32).T  # [256, 256]
    W1aug = np.zeros((IN_FEATS, 262), np.float32)
    W1aug[:, 0:128] = W1t[:, 0:128]
    W1aug[:, 129:257] = W1t[:, 128:256]
    a_s, a_d = np.asarray(att_src1, np.float32), np.asarray(att_dst1, np.float32)
    for k in range(2):
        W1aug[:, 258 + k] = W1t[:, k * 128:(k + 1) * 128] @ a_s[0, k]
        W1aug[:, 260 + k] = W1t[:, k * 128:(k + 1) * 128] @ a_d[0, k]
    W2t = np.asarray(W2, np.float32).T  # [256, 128]
    W2aug = np.zeros((IN_FEATS, 131), np.float32)
    W2aug[:, 0:128] = W2t
    W2aug[:, 129] = W2t @ np.asarray(att_src2, np.float32)[0, 0]
    W2aug[:, 130] = W2t @ np.asarray(att_dst2, np.float32)[0, 0]
    return W1aug.astype(F16), W2aug.astype(F16)


# --------------------------------------------------------------------------
# Device program
# --------------------------------------------------------------------------

def _build_program(n_nodes, plan, phases=6):
    """phases: 1=dense1, 2=+ag1, 3=+edge1+dense2, 5=+ag2, 6=full"""
    import concourse.bass as bass
    import concourse.bacc as bacc
    import concourse.mybir as mybir
    import concourse.tile as tile

    dt = mybir.dt
    nsh, nblk, T = plan["nsh"], plan["nblk"], plan["T"]
    tiles_bs, blk_start = plan["tiles_bs"], plan["blk_start"]
    strm, stream_pos, t_sizes = plan["strm"], plan["stream_pos"], plan["t_sizes"]
    npad = nblk * P

    nc = bacc.Bacc("TRN2", target_bir_lowering=False, debug=False,
                   enable_asserts=True, num_devices=N_CORES,
                   num_swdge_queues=int(_os.environ.get("K_QUEUES", "4")))

    # ---- I/O ----
    xT = nc.dram_tensor("xT", [IN_FEATS, npad], dt.float16, kind="ExternalInput")
    w1 = nc.dram_tensor("W1aug", [IN_FEATS, 262], dt.float16, kind="ExternalInput")
    w2 = nc.dram_tensor("W2aug", [IN_FEATS, 131], dt.float16, kind="ExternalInput")
    NS = NCHUNK + 1
    gsrc_d = [nc.dram_tensor(f"gsrc{s}", [P, max(1, 8 * t_sizes[s])], dt.int16,
                             kind="ExternalInput") for s in range(NS)]
    adsti_d = [nc.dram_tensor(f"adsti{s}", [P, max(1, 8 * t_sizes[s])], dt.int16,
                              kind="ExternalInput") for s in range(NS)]
    dstcol_d = nc.dram_tensor("dstcol", [P, T], dt.float32, kind="ExternalInput")
    out_d = nc.dram_tensor("out", [nsh, HIDDEN], dt.float32, kind="ExternalOutput")
    import os
    dbg = int(os.environ.get("K_DEBUG", "0"))
    if dbg:
        dbg_h1 = nc.dram_tensor("dbg_h1", [nsh, ROW1], dt.float16, kind="ExternalOutput")
        dbg_o1 = nc.dram_tensor("dbg_o1", [nsh, 256], dt.float16, kind="ExternalOutput")
        dbg_ad1 = nc.dram_tensor("dbg_ad1", [nsh, 128], dt.float16, kind="ExternalOutput")
        dbg_h2 = nc.dram_tensor("dbg_h2", [nsh, 130], dt.float16, kind="ExternalOutput")

    # ---- internal DRAM ----
    h1_slab = nc.dram_tensor("h1_slab", [nsh, ROW1], dt.float16)
    h1p = nc.dram_tensor("h1p", [nsh, 260], dt.float16)  # packed cc1 input
    tab1p = [nc.dram_tensor(f"tab1p_{j}", [N_CORES * CSZ, 260], dt.float16,
                            addr_space="Shared") for j in range(NCHUNK)]
    tab1 = [nc.dram_tensor(f"tab1_{j}", [N_CORES * CSZ, ROW1], dt.float16,
                           addr_space="Shared") for j in range(NCHUNK)]
    ad1 = nc.dram_tensor("ad1", [nsh, 128], dt.float16)  # fp32 pairs, bitcast
    h2_slab = nc.dram_tensor("h2_slab", [nsh, ROW2], dt.float16)
    h2p = nc.dram_tensor("h2p", [nsh, 130], dt.float16)  # packed cc2 input
    tab2p = [nc.dram_tensor(f"tab2p_{j}", [N_CORES * CSZ, 130], dt.float16,
                            addr_space="Shared") for j in range(NCHUNK)]
    tab2 = [nc.dram_tensor(f"tab2_{j}", [N_CORES * CSZ, ROW2], dt.float16)
            for j in range(NCHUNK)]
    ad2 = nc.dram_tensor("ad2", [nsh, 128], dt.float16)

    groups = [list(range(N_CORES))]

    with tile.TileContext(nc) as tc:
        import contextlib
        ctx = contextlib.ExitStack()
        with ctx:
            res = ctx.enter_context(tc.tile_pool(name="res", bufs=1))
            dense_ps = ctx.enter_context(tc.tile_pool(name="dps", bufs=2, space="PSUM"))
            dense_sb = ctx.enter_context(tc.tile_pool(name="dsb", bufs=3))
            gath = ctx.enter_context(tc.tile_pool(name="gath", bufs=4))
            alph = ctx.enter_context(tc.tile_pool(name="alph", bufs=2))
            sal = ctx.enter_context(tc.tile_pool(name="sal", bufs=6))
            blk_ps = ctx.enter_context(tc.tile_pool(name="bps", bufs=2, space="PSUM"))
            tr_ps = ctx.enter_context(tc.tile_pool(name="tps", bufs=1, space="PSUM"))
            epi = ctx.enter_context(tc.tile_pool(name="epi", bufs=3))

            # ---- resident tiles ----
            # xT is dead after dense1; o1T reuses the same resident tiles
            # (WAR deps keep dense1's reads ahead of epi1's writes)
            xT_sb = [res.tile([P, npad], dt.float16, tag=f"xT{k}", name=f"xT{k}")
                     for k in range(2)]
            w1_sb = [res.tile([P, 262], dt.float16, tag=f"w1_{k}", name=f"w1_{k}")
                     for k in range(2)]
            w2_sb = [res.tile([P, 131], dt.float16, tag=f"w2_{k}", name=f"w2_{k}")
                     for k in range(2)]
            gsrc_sb = [res.tile([P, max(1, 8 * t_sizes[s])], dt.int16,
                                tag=f"gsrc{s}", name=f"gsrc{s}") for s in range(NS)]
            adsti_sb = [res.tile([P, max(1, 8 * t_sizes[s])], dt.int16,
                                 tag=f"adsti{s}", name=f"adsti{s}") for s in range(NS)]
            dstcol_sb = res.tile([P, T], dt.float32, tag="dstcol", name="dstcol")
            iota_i = res.tile([P, P], dt.int16, tag="iota_i", name="iota_i")
            iota_f = res.tile([P, P], dt.float16, tag="iota_f", name="iota_f")
            iota_p = res.tile([P, 1], dt.int16, tag="iota_p", name="iota_p")
            iota_pf = res.tile([P, 1], dt.float32, tag="iota_pf", name="iota_pf")
            ident = res.tile([P, P], dt.float16, tag="ident", name="ident")
            o1T_sb = xT_sb
            nshift = res.tile([P, 1], dt.float32, tag="nshift", name="nshift")
            # per-block phase-A partials: [p, nblk, 2*129] fp16 (L1 uses both
            # halves; L2 reuses [:, :, 0:129])
            part = res.tile([P, nblk, 258], dt.float16, tag="part", name="part")

            for k in range(2):
                nc.sync.dma_start(out=xT_sb[k][:], in_=xT[k * P:(k + 1) * P, :])
                nc.sync.dma_start(out=w1_sb[k][:], in_=w1[k * P:(k + 1) * P, :])
                nc.sync.dma_start(out=w2_sb[k][:], in_=w2[k * P:(k + 1) * P, :])
            for s in range(NS):
                nc.sync.dma_start(out=gsrc_sb[s][:], in_=gsrc_d[s][:, :])
                nc.sync.dma_start(out=adsti_sb[s][:], in_=adsti_d[s][:, :])
            nc.sync.dma_start(out=dstcol_sb[:], in_=dstcol_d[:, :])
            nc.gpsimd.iota(iota_i[:], pattern=[[1, P]], channel_multiplier=0)
            nc.vector.tensor_copy(out=iota_f[:], in_=iota_i[:])
            nc.gpsimd.iota(iota_p[:], pattern=[[0, 1]], channel_multiplier=1)
            nc.vector.tensor_copy(out=iota_pf[:], in_=iota_p[:])
            # identity[p, j] = (j == p)
            nc.vector.tensor_scalar(
                out=ident[:], in0=iota_f[:], scalar1=iota_pf[:, 0:1],
                scalar2=None, op0=mybir.AluOpType.is_equal)
            nc.vector.memset(nshift[:], -SHIFT)
            # o1T pad columns [nsh:npad) stay zero from the zero-padded xT load

            # ---------------- Layer 1 dense ----------------
            def dense1():
                writes = []  # per-block write handles
                for nb in range(nblk):
                    rows = min(P, nsh - nb * P)
                    ps = dense_ps.tile([P, 262], dt.float32, tag="dps", name="dps")
                    for kc in range(2):
                        nc.tensor.matmul(
                            ps[:], lhsT=xT_sb[kc][:, nb * P:(nb + 1) * P],
                            rhs=w1_sb[kc][:], start=(kc == 0), stop=(kc == 1))
                    stg = dense_sb.tile([P, ROW1], dt.float16, tag="dstg", name="dstg")
                    nc.vector.tensor_copy(out=stg[:, 0:260], in_=ps[:, 0:260])
                    nc.vector.memset(stg[:, 128:129], 1.0)
                    nc.vector.memset(stg[:, 257:258], 1.0)
                    nc.vector.memset(stg[:, 260:ROW1], 0.0)
                    astg = dense_sb.tile([P, 64], dt.float32, tag="astg", name="astg")
                    nc.vector.memset(astg[:], 0.0)
                    nc.scalar.activation(out=astg[:, 0:2], in_=ps[:, 260:262],
                                         func=mybir.ActivationFunctionType.Copy)
                    blkw = [nc.sync.dma_start(
                        out=h1_slab[nb * P:nb * P + rows, :], in_=stg[:rows, :]),
                            nc.scalar.dma_start(
                        out=ad1[nb * P:nb * P + rows, :],
                        in_=astg[:rows, 0:64].bitcast(dt.float16))]
                    if PACK1:
                        blkw.append(nc.sync.dma_start(
                            out=h1p[nb * P:nb * P + rows, :],
                            in_=stg[:rows, 0:260]))
                    writes.append(blkw)
                    if dbg:
                        nc.sync.dma_start(out=dbg_h1[nb * P:nb * P + rows, :],
                                          in_=stg[:rows, :])
                        nc.sync.dma_start(out=dbg_ad1[nb * P:nb * P + rows, :],
                                          in_=astg[:rows, 0:64].bitcast(dt.float16))
                return writes

            def chunk_ccs(slab, tabs, blk_writes):
                """Two AllGathers over the slab halves; each depends on the
                dense writes of the blocks overlapping its row range."""
                from bass_rust import add_dep_helper
                ccs = []
                for j in range(NCHUNK):
                    r0, r1 = j * CSZ, (j + 1) * CSZ
                    cc = nc.gpsimd.collective_compute(
                        "AllGather", mybir.AluOpType.bypass,
                        replica_groups=groups,
                        ins=[slab[r0:r1, :]], outs=[tabs[j].ap()])
                    for b in range(r0 // P, -(-r1 // P)):
                        if b < len(blk_writes):
                            for w in blk_writes[b]:
                                add_dep_helper(cc.ins, w.ins, sync=True,
                                               reason="allgather after dense writes")
                    ccs.append(cc)
                return ccs

            def edge_layer(tabs, adt, heads, rowlen, asrc_off, epilogue, deps,
                           on_block=None):
                """tabs[s]: gather table per stream (s=0 local slab).
                deps[s]: instruction handles each stream's gathers wait on."""
                from bass_rust import add_dep_helper
                psum = {}
                for s in range(NS):
                    sbuf = None  # current (gbuf, ale)
                    for sp in range(t_sizes[s]):
                        t = stream_t[s][sp]
                        g, j = divmod(sp, G_TILES)
                        if j == 0:
                            gw = min(G_TILES, t_sizes[s] - g * G_TILES)
                            nq = nc.num_swdge_queues
                            gbuf = gath.tile([P, gw, rowlen], dt.float16,
                                             tag="gbuf", name=f"gbuf{s}")
                            gi = nc.gpsimd.dma_gather(
                                out_ap=gbuf[:], in_ap=tabs[s][:, :],
                                idxs_ap=gsrc_sb[s][:, g * G_TILES * 8:(g * G_TILES + gw) * 8],
                                num_idxs=gw * P, num_idxs_reg=gw * P,
                                elem_size=rowlen, queue_num=(2 * g) % nq)
                            abuf = gath.tile([P, gw, 128], dt.float16,
                                             tag="abuf", name=f"abuf{s}")
                            ai = nc.gpsimd.dma_gather(
                                out_ap=abuf[:], in_ap=adt[:, :],
                                idxs_ap=adsti_sb[s][:, g * G_TILES * 8:(g * G_TILES + gw) * 8],
                                num_idxs=gw * P, num_idxs_reg=gw * P,
                                elem_size=128, queue_num=(2 * g + 1) % nq)
                            for d in deps[s]:
                                add_dep_helper(gi.ins, d.ins, sync=True,
                                               reason="gather after table ready")
                                add_dep_helper(ai.ins, d.ins, sync=True,
                                               reason="adst gather after table ready")
                            # alpha = exp(lrelu(asrc + adst) - SHIFT)
                            asr = alph.tile([P, gw, heads], dt.float32,
                                            tag=f"asr{s}", name=f"asr{s}")
                            nc.vector.tensor_copy(
                                out=asr[:], in_=gbuf[:, :, asrc_off:asrc_off + heads])
                            tsum = alph.tile([P, gw, heads], dt.float32,
                                             tag=f"tsum{s}", name=f"tsum{s}")
                            nc.vector.tensor_tensor(
                                out=tsum[:], in0=asr[:],
                                in1=abuf[:, :, 0:2 * heads].bitcast(dt.float32),
                                op=mybir.AluOpType.add)
                            tng = alph.tile([P, gw, heads], dt.float32,
                                            tag=f"tng{s}", name=f"tng{s}")
                            nc.vector.tensor_scalar(
                                out=tng[:], in0=tsum[:], scalar1=NEG_SLOPE,
                                scalar2=None, op0=mybir.AluOpType.mult)
                            lr = alph.tile([P, gw, heads], dt.float32,
                                           tag=f"lr{s}", name=f"lr{s}")
                            nc.vector.tensor_tensor(
                                out=lr[:], in0=tsum[:], in1=tng[:],
                                op=mybir.AluOpType.max)
                            ale = alph.tile([P, gw, heads], dt.float32,
                                            tag=f"ale{s}", name=f"ale{s}")
                            nc.scalar.activation(
                                out=ale[:], in_=lr[:],
                                func=mybir.ActivationFunctionType.Exp,
                                bias=nshift[:])
                            sbuf = (gbuf, ale)
                        gbuf, ale = sbuf
                        b = int(np.searchsorted(blk_start, t, side="right")) - 1
                        first = sp == blk_sp0[s][b]
                        last = sp == blk_sp0[s][b] + tiles_bs[b, s] - 1
                        if first:
                            psum = {h: blk_ps.tile([P, 129], dt.float32,
                                                   tag=f"pb{h}", name=f"pb{h}")
                                    for h in range(heads)}
                        for h in range(heads):
                            sa = sal.tile([P, P], dt.float16, tag=f"sa{h}",
                                          name=f"sa{h}")
                            nc.vector.tensor_scalar(
                                out=sa[:], in0=iota_f[:],
                                scalar1=dstcol_sb[:, t:t + 1],
                                scalar2=ale[:, j, h:h + 1],
                                op0=mybir.AluOpType.is_equal,
                                op1=mybir.AluOpType.mult)
                            nc.tensor.matmul(
                                out=psum[h][:], lhsT=sa[:],
                                rhs=gbuf[:, j, 129 * h:129 * h + 129],
                                start=first, stop=last)
                        if last:
                            if s == 0:
                                # flush local-phase partial to SBUF (fp16)
                                for h in range(heads):
                                    nc.scalar.activation(
                                        out=part[:, b, h * 129:(h + 1) * 129],
                                        in_=psum[h][:],
                                        func=mybir.ActivationFunctionType.Copy)
                            elif s < NS - 1:
                                # accumulate phase partial into SBUF
                                for h in range(heads):
                                    nc.vector.tensor_tensor(
                                        out=part[:, b, h * 129:(h + 1) * 129],
                                        in0=psum[h][:],
                                        in1=part[:, b, h * 129:(h + 1) * 129],
                                        op=mybir.AluOpType.add)
                            else:
                                epilogue(b, psum)
                                if on_block is not None:
                                    on_block(b)

            def bail():
                dummy = epi.tile([P, HIDDEN], dt.float32, tag="dummy", name="dummy")
                nc.vector.memset(dummy[:], 0.0)
                for nb in range(nblk):
                    rows = min(P, nsh - nb * P)
                    nc.scalar.dma_start(out=out_d[nb * P:nb * P + rows, :],
                                        in_=dummy[:rows, :])

            # precomputed stream->tile maps
            stream_t = [np.nonzero(strm == s)[0] for s in range(NS)]
            blk_sp0 = []  # per stream: first stream_pos of each block
            for s in range(NS):
                sp0 = np.zeros(nblk, np.int64)
                acc = 0
                for b in range(nblk):
                    sp0[b] = acc
                    acc += tiles_bs[b, s]
                blk_sp0.append(sp0)

            # ---------------- Layer 1 ----------------
            from bass_rust import add_dep_helper as _adh
            d1w = dense1()
            cc1s = None
            if phases >= 2:
                if PACK1:
                    cc1s = chunk_ccs(h1p, tab1p, d1w)
                    # repack 520B-pitch rows into the 768B-pitch gather table
                    rps1 = []
                    for j in range(NCHUNK):
                        rp = nc.sync.dma_start(out=tab1[j][:, 0:260],
                                               in_=tab1p[j][:, :])
                        _adh(rp.ins, cc1s[j].ins, sync=True,
                             reason="repack after allgather1")
                        rps1.append(rp)
                    cc1s = rps1
                else:
                    cc1s = chunk_ccs(h1_slab, tab1, d1w)

            d2_writes = [[] for _ in range(nblk)]

            def epi1(b, psum):
                """Merge partials, divide+ReLU, PE-transpose into o1T, then
                layer-2 dense for this block."""
                rows = min(P, nsh - b * P)
                mg = {}
                for h in range(2):
                    m = epi.tile([P, 129], dt.float32, tag=f"mg{h}", name=f"mg{h}")
                    nc.vector.tensor_tensor(
                        out=m[:], in0=psum[h][:],
                        in1=part[:, b, h * 129:(h + 1) * 129],
                        op=mybir.AluOpType.add)
                    mg[h] = m
                rc = epi.tile([P, 2], dt.float32, tag="rc", name="rc")
                dn = epi.tile([P, 2], dt.float32, tag="dn", name="dn")
                for h in range(2):
                    nc.vector.tensor_scalar(
                        out=dn[:, h:h + 1], in0=mg[h][:, 128:129], scalar1=1e-6,
                        scalar2=None, op0=mybir.AluOpType.max)
                nc.vector.reciprocal(out=rc[:], in_=dn[:])
                o1s = epi.tile([P, 256], dt.float16, tag="o1s", name="o1s")
                for h in range(2):
                    nc.scalar.activation(
                        out=o1s[:, h * 128:(h + 1) * 128], in_=mg[h][:, 0:128],
                        func=mybir.ActivationFunctionType.Relu,
                        scale=rc[:, h:h + 1])
                if dbg:
                    nc.scalar.dma_start(out=dbg_o1[b * P:b * P + rows, :],
                                        in_=o1s[:rows, :])
                # transpose into resident lhsT for layer 2
                for k in range(2):
                    tp = tr_ps.tile([P, P], dt.float16, tag="tp", name="tp")
                    nc.tensor.transpose(tp[:], o1s[:, k * 128:(k + 1) * 128],
                                        ident[:])
                    nc.scalar.activation(
                        out=o1T_sb[k][:, b * P:b * P + rows],
                        in_=tp[:, 0:rows],
                        func=mybir.ActivationFunctionType.Copy)
                # layer-2 dense for this block
                ps2 = dense_ps.tile([P, 262], dt.float32, tag="dps", name="dps2")
                for kc in range(2):
                    nc.tensor.matmul(
                        ps2[:, 0:131], lhsT=o1T_sb[kc][:, b * P:(b + 1) * P],
                        rhs=w2_sb[kc][:], start=(kc == 0), stop=(kc == 1))
                stg = dense_sb.tile([P, 130], dt.float16, tag="dstg2", name="dstg2")
                nc.scalar.activation(out=stg[:, 0:130], in_=ps2[:, 0:130],
                                     func=mybir.ActivationFunctionType.Copy)
                nc.vector.memset(stg[:, 128:129], 1.0)
                astg = dense_sb.tile([P, 64], dt.float32, tag="astg2", name="astg2")
                nc.vector.memset(astg[:], 0.0)
                nc.scalar.activation(out=astg[:, 0:1], in_=ps2[:, 130:131],
                                     func=mybir.ActivationFunctionType.Copy)
                d2_writes[b].append(nc.sync.dma_start(
                    out=h2_slab[b * P:b * P + rows, 0:130], in_=stg[:rows, :]))
                if PACK2:
                    d2_writes[b].append(nc.sync.dma_start(
                        out=h2p[b * P:b * P + rows, :], in_=stg[:rows, :]))
                d2_writes[b].append(nc.scalar.dma_start(
                    out=ad2[b * P:b * P + rows, :],
                    in_=astg[:rows, 0:64].bitcast(dt.float16)))
                if dbg:
                    nc.sync.dma_start(out=dbg_h2[b * P:b * P + rows, :],
                                      in_=stg[:rows, :])

            # cc2 chunks are launched mid-edge1, as soon as the last block of
            # each h2_slab half has been written, so the collective overlaps
            # the remainder of edge1's phase B.
            cc2s = [None, None]

            def launch_cc2(j):
                from bass_rust import add_dep_helper
                r0, r1 = j * CSZ, (j + 1) * CSZ
                if PACK2:
                    cc = nc.gpsimd.collective_compute(
                        "AllGather", mybir.AluOpType.bypass,
                        replica_groups=groups,
                        ins=[h2p[r0:r1, :]], outs=[tab2p[j].ap()])
                else:
                    cc = nc.gpsimd.collective_compute(
                        "AllGather", mybir.AluOpType.bypass,
                        replica_groups=groups,
                        ins=[h2_slab[r0:r1, :]], outs=[tab2[j].ap()])
                for b in range(r0 // P, -(-r1 // P)):
                    for w in d2_writes[b]:
                        add_dep_helper(cc.ins, w.ins, sync=True,
                                       reason="allgather2 after dense writes")
                cc2s[j] = cc

            cc2_blocks = {(-(-((j + 1) * CSZ) // P)) - 1: j for j in range(NCHUNK)}

            def on_block1(b):
                if phases >= 5 and b in cc2_blocks:
                    launch_cc2(cc2_blocks[b])

            if phases < 3:
                bail()
            if phases >= 3:
                d1flat = [w for blk in d1w for w in blk]
                edge_layer([h1_slab] + tab1, ad1, 2, ROW1, 258, epi1,
                           [d1flat, [cc1s[0]], [cc1s[1]]], on_block=on_block1)

            def epi2(b, psum):
                rows = min(P, nsh - b * P)
                mg = epi.tile([P, 129], dt.float32, tag="mg2", name="mg2")
                nc.vector.tensor_tensor(
                    out=mg[:], in0=psum[0][:], in1=part[:, b, 0:129],
                    op=mybir.AluOpType.add)
                rc = epi.tile([P, 1], dt.float32, tag="rc2", name="rc2")
                dn = epi.tile([P, 1], dt.float32, tag="dn2", name="dn2")
                nc.vector.tensor_scalar(
                    out=dn[:], in0=mg[:, 128:129], scalar1=1e-6,
                    scalar2=None, op0=mybir.AluOpType.max)
                nc.vector.reciprocal(out=rc[:], in_=dn[:])
                os_ = epi.tile([P, 128], dt.float32, tag="os", name="os")
                nc.scalar.activation(
                    out=os_[:], in_=mg[:, 0:128],
                    func=mybir.ActivationFunctionType.Copy, scale=rc[:, 0:1])
                nc.scalar.dma_start(out=out_d[b * P:b * P + rows, :],
                                    in_=os_[:rows, :])

            if phases >= 6:
                from bass_rust import add_dep_helper as _adh2
                # repack 260B-pitch collective outputs into 512B-pitch gather
                # tables; emitted after all edge1 SP writes so the wait on the
                # collective cannot block the slab-write pipeline
                rps = []
                for j in range(NCHUNK):
                    if PACK2:
                        rp = nc.sync.dma_start(out=tab2[j][:, 0:130],
                                               in_=tab2p[j][:, :])
                        _adh2(rp.ins, cc2s[j].ins, sync=True,
                              reason="repack after allgather2")
                    else:
                        rp = cc2s[j]
                    rps.append(rp)
                d2flat = [w for blk in d2_writes for w in blk]
                edge_layer([h2_slab] + tab2, ad2, 1, ROW2, 129, epi2,
                           [d2flat, [rps[0]], [rps[1]]])
            elif phases >= 3:
                bail()

    nc.compile()
    return nc


# --------------------------------------------------------------------------
# Host entry
# --------------------------------------------------------------------------

def _make_in_maps(inputs, plan):
    x = np.asarray(inputs["x"], np.float32)
    W1aug, W2aug = _prep_weights(
        inputs["W1"], inputs["att_src1"], inputs["att_dst1"],
        inputs["W2"], inputs["att_src2"], inputs["att_dst2"])
    nsh, nblk = plan["nsh"], plan["nblk"]
    npad = nblk * P
    in_maps = []
    for c in range(N_CORES):
        xs = x[c * nsh:(c + 1) * nsh]
        xT = np.zeros((IN_FEATS, npad), F16)
        xT[:, :nsh] = xs.T.astype(F16)
        m = {"xT": xT, "W1aug": W1aug, "W2aug": W2aug, "dstcol": plan["dstcol"][c]}
        for s in range(NCHUNK + 1):
            gw = plan["gsrc_w"][s][c]
            aw = plan["adsti_w"][s][c]
            if gw.shape[1] == 0:
                gw = np.zeros((P, 1), np.int16)
                aw = np.zeros((P, 1), np.int16)
            m[f"gsrc{s}"] = gw
            m[f"adsti{s}"] = aw
        in_maps.append(m)
    return in_maps


def run(inputs, trace=False, **spmd_kwargs):
    assert float(np.abs(np.asarray(inputs["b1"])).max()) == 0.0, "b1 must be 0"
    plan = _plan_edges(inputs["edge_index"], N_NODES)
    import os
    nc = _build_program(N_NODES, plan,
                        phases=int(os.environ.get("K_PHASES", "6")))
    in_maps = _make_in_maps(inputs, plan)
    from concourse import bass_utils
    res = bass_utils.run_bass_kernel_spmd(
        nc, in_maps, core_ids=list(range(N_CORES)), trace=trace, **spmd_kwargs)
    out = np.concatenate([res.results[c]["out"] for c in range(N_CORES)], axis=0)
    out = (out + np.asarray(inputs["b2"], np.float32)[None, :]).astype(np.float32)
    return out, res


def kernel(**inputs):
    return run(inputs)[0]
